# revision 1
# baseline (speedup 1.0000x reference)
"""Trainium2 Bass kernel for the BalSCL/SSL balanced supervised-contrastive loss.

fp8 redesign of the bf16 baseline (97us); measures ~67-68us on HW:

  * Raw logits matmul in fp8-e4m3 DoubleRow; K=D=128 only, so the second
    k-tile streams a zero block (X1=0, W1=garbage/I).  exp stored as
    fp8-e5m2; the per-class accumulation (E-matmul) pairs two j-tiles per
    DoubleRow pass with both k-tiles real, halving its matmul count.
  * exp computed on BOTH the scalar engine (true Exp LUT -> e5m2) and the
    vector engine via the Schraudolph trick: e5m2 bit pattern of 2^t is the
    integer 4*t + 60, so  int8 = (raw * A) + B  with A = 4*log2(e)/TEMP,
    B ~ 60 directly synthesizes exp(raw/TEMP) in e5m2.  B = 59.75 calibrated
    for the HW convert (round-to-nearest; CoreSim truncates, so its loss
    differs slightly).  Groups strictly alternate ACT/DVE so neither exp
    engine ever serves back-to-back groups.

    Measured HW facts this design is built around: a 512-col matmul costs
    ~215ns at full p-state regardless of dtype/perf-mode (DoubleRow doubles
    K, not column rate), so fp8-DR only pays where both k-tiles are real
    (the E-matmul); dual-fp8 LDWEIGHTS needs a 16-byte-aligned k-tile
    stride (hence CEP=112); GPSIMD cannot touch PSUM; the scalar engine is
    (172+cols)/1.2GHz per activation; the vector engine (120+cols)/0.96GHz.
  * The self-contrast diagonal is killed pre-exp: a tiny extra matmul adds
    -2 to raw_ii (exp(10*(1-2)) ~ 5e-5, negligible), so no exp-matching
    diag correction is needed anywhere.  Each core's own 8 diagonal j-tiles
    are permuted to positions 0..7 by the host so the fixup is static in the
    shared instruction stream.
  * Classes extended to 101: class 100 ("conf class") is hit by exactly one
    zero-padding row whose exp is exactly 1.0, and W2C row 100 = (1-conf),
    so the weighted column-sum directly yields S' = conf*S + (1-conf) and
    ln-accum gives numA = sum(conf*ln S) with no extra elementwise fixups.
  * Everything small (Sm positive-pair sums, 1/m, class counts, final
    num/den combine) moved to the host: the device ships the raw per-row
    ln(S') vector [1, 1024] and the host sums it (conf=0 rows are ln(1)=0).
"""

import os
import sys

sys.path.insert(0, "/opt/trn_rl_repo")

import numpy as np
import ml_dtypes

import concourse.bass as bass  # noqa: F401
import concourse.bacc as bacc
import concourse.tile as tile
from concourse import mybir
from concourse.bass_utils import run_bass_kernel_spmd

F32 = mybir.dt.float32
BF16 = mybir.dt.bfloat16
E4 = mybir.dt.float8e4
E5 = mybir.dt.float8e5
I8 = mybir.dt.int8
BF = ml_dtypes.bfloat16
E4NP = ml_dtypes.float8_e4m3
E5NP = ml_dtypes.float8_e5m2
AF = mybir.ActivationFunctionType
ALU = mybir.AluOpType
PM = mybir.MatmulPerfMode

B2, C, D = 8192, 100, 128
CE = C + 1                # 101: class 100 is the "conf class"
CEP = 112                 # TAg per-tile column stride (dual-fp8 LW needs %16==0)
TEMP = 0.1
N = B2 + C                # 8292 real columns
TJ = 66                   # j-tiles (NPAD = 8448; rows >= 8293 zero padding)
NPAD = TJ * 128
CORES = 8
R = B2 // CORES           # 1024 rows per core
CH = 512                  # i-chunk width
NG = TJ // 2              # 33 pair-groups per chunk
A_SLOPE = 4.0 * (1.0 / TEMP) * float(np.log2(np.e))   # 57.7078


def fg_off(t):
    """fTg column offset of tile t: an I-block follows each of tiles 0..7."""
    return 256 * t if t < 8 else 1024 + 128 * t


FGW = fg_off(TJ) + 256  # one garbage W1 block after the last tile
B_CAL = float(os.environ.get("KB_BCAL", "59.75"))  # HW f32->int8 convert rounds
N_WARM = int(os.environ.get("KB_WARM", "22"))

# groups handled by the vector engine (Schraudolph); rest by ACT Exp LUT.
# Strict alternation: the processed order also alternates parity, so ACT/DVE
# take turns and neither stalls the PE pipeline with back-to-back service.
_DVE_PAT = os.environ.get("KB_DVEPAT", "odd")
if os.environ.get("KB_DVE", "1") != "1":
    DVE_G = set()
elif _DVE_PAT == "odd":
    DVE_G = {g for g in range(NG) if g % 2 == 1}
else:
    DVE_G = {g for g in range(NG) if (g % 11) in {int(ch) for ch in _DVE_PAT}}

FLAG_ONETAB = os.environ.get("KB_ONETAB", "1") == "1"

_NC_CACHE = {}

# Combined exp+ln activation-table set: a single ACT_TABLE_LOAD.
_orig_gat = bacc.get_activation_tables


def _gat_combined(arch):
    tabs = _orig_gat(arch)
    if not FLAG_ONETAB:
        return tabs
    out = {}
    for name, funcs in tabs.items():
        if name in ("exp_and_others", "exp_and_friends", "natural_log"):
            out[name] = set()  # keep position (set ids are positional)
        else:
            out[name] = funcs
    return out


def _build_nc():
    bacc.get_activation_tables = _gat_combined
    try:
        return _build_nc_inner()
    finally:
        bacc.get_activation_tables = _orig_gat


def _build_nc_inner():
    nc = bacc.Bacc()

    fTg = nc.dram_tensor("fTg", [128, FGW], E4, kind="ExternalInput")
    fTcz = nc.dram_tensor("fTcz", [128, 10240], E4, kind="ExternalInput")
    TAg = nc.dram_tensor("TAg", [128, TJ * CEP], E5, kind="ExternalInput")
    W2C = nc.dram_tensor("W2C", [CE, R], BF16, kind="ExternalInput")
    outd = nc.dram_tensor("out", [1, 2 * CH], F32, kind="ExternalOutput")

    with tile.TileContext(nc) as tc:
        with (
            tc.tile_pool(name="consts", bufs=1) as cp,
            tc.tile_pool(name="expp", bufs=6) as ep,
            tc.tile_pool(name="asmp", bufs=2) as am,
            tc.tile_pool(name="rawp", bufs=3, space="PSUM") as rp,
            tc.tile_pool(name="epsp", bufs=2, space="PSUM") as pp,
            tc.tile_pool(name="outp", bufs=1) as op,
        ):
            # ---------------- input loads (ordered by first use) ------------
            s_fTcz = cp.tile([128, 10240], E4)
            s_fTg = cp.tile([128, FGW], E4)
            s_TAg = cp.tile([128, TJ * CEP], E5)
            s_W2C = cp.tile([CE, R], BF16)

            # chunk0 runs groups [2..32, 0, 1]: critical path needs the c0
            # rhs slot, fTg tiles 4.. (with their W1 neighbours), TAg from
            # group 2 on.  Diag slots / tiles 0-3 / TAg[:448] arrive later.
            # criticals first on BOTH desc engines; memsets after gpsimd's
            nc.sync.dma_start(out=s_fTcz[:, 0:256], in_=fTcz[:, 0:256])
            nc.gpsimd.dma_start(out=s_fTcz[:, 256:512], in_=fTcz[:, 256:512])
            nc.sync.dma_start(out=s_fTg[:, 1024:1280], in_=fTg[:, 1024:1280])
            nc.gpsimd.dma_start(out=s_fTg[:, 1280:1536], in_=fTg[:, 1280:1536])
            nc.gpsimd.memset(s_fTcz[:, 512:1024], 0.0)
            nc.gpsimd.memset(s_fTcz[:, 1536:2048], 0.0)
            nc.sync.dma_start(out=s_fTg[:, 1536:2048], in_=fTg[:, 1536:2048])
            nc.gpsimd.dma_start(
                out=s_TAg[:, 4 * CEP : 10 * CEP], in_=TAg[:, 4 * CEP : 10 * CEP]
            )
            nc.sync.dma_start(out=s_fTg[:, 2048:2560], in_=fTg[:, 2048:2560])
            nc.gpsimd.dma_start(out=s_fTg[:, 2560:3072], in_=fTg[:, 2560:3072])
            nc.sync.dma_start(out=s_fTg[:, 3072:3584], in_=fTg[:, 3072:3584])
            nc.gpsimd.dma_start(
                out=s_TAg[:, 10 * CEP : 18 * CEP], in_=TAg[:, 10 * CEP : 18 * CEP]
            )
            nc.sync.dma_start(out=s_fTg[:, 3584:4608], in_=fTg[:, 3584:4608])
            nc.gpsimd.dma_start(out=s_fTg[:, 4608:5632], in_=fTg[:, 4608:5632])
            nc.sync.dma_start(
                out=s_TAg[:, 18 * CEP : 28 * CEP], in_=TAg[:, 18 * CEP : 28 * CEP]
            )
            nc.gpsimd.dma_start(out=s_fTg[:, 5632:6656], in_=fTg[:, 5632:6656])
            nc.sync.dma_start(out=s_fTg[:, 6656:7680], in_=fTg[:, 6656:7680])
            nc.gpsimd.dma_start(
                out=s_TAg[:, 28 * CEP : 40 * CEP], in_=TAg[:, 28 * CEP : 40 * CEP]
            )
            nc.sync.dma_start(out=s_fTg[:, 7680:FGW], in_=fTg[:, 7680:FGW])
            nc.gpsimd.dma_start(out=s_fTcz[:, 2048:4096], in_=fTcz[:, 2048:4096])
            nc.sync.dma_start(
                out=s_TAg[:, 40 * CEP : 54 * CEP], in_=TAg[:, 40 * CEP : 54 * CEP]
            )
            nc.gpsimd.dma_start(out=s_fTcz[:, 4096:6144], in_=fTcz[:, 4096:6144])
            nc.sync.dma_start(out=s_fTg[:, 0:1024], in_=fTg[:, 0:1024])
            nc.gpsimd.dma_start(out=s_TAg[:, 54 * CEP :], in_=TAg[:, 54 * CEP :])
            nc.sync.dma_start(out=s_TAg[:, 0 : 4 * CEP], in_=TAg[:, 0 : 4 * CEP])
            nc.gpsimd.dma_start(out=s_W2C, in_=W2C[:])
            nc.sync.dma_start(out=s_fTcz[:, 1024:1536], in_=fTcz[:, 1024:1536])
            nc.gpsimd.dma_start(out=s_fTcz[:, 6144:8192], in_=fTcz[:, 6144:8192])
            nc.sync.dma_start(out=s_fTcz[:, 8192:10240], in_=fTcz[:, 8192:10240])

            s_ones = cp.tile([CE, 1], BF16)
            nc.vector.memset(s_ones, 1.0)
            s_scr = cp.tile([128, 256], BF16)
            nc.vector.memset(s_scr, 1.0)

            # PE warm-up in the DMA-wait window (HAM un-throttle)
            warmPS = pp.tile([128, 256], F32, name="warmPS", tag="EPS")
            for _ in range(N_WARM):
                nc.tensor.matmul(
                    warmPS, lhsT=s_scr[:, 0:128], rhs=s_scr, start=True, stop=True
                )

            outsb = op.tile([1, 2 * CH], F32)

            def mk_w2e(c, EPS, box):
                def go():
                    W2E = am.tile([CE, CH], BF16, name=f"W2E{c}", tag="W2E")
                    nc.vector.tensor_mul(W2E, EPS, s_W2C[:, CH * c : CH * (c + 1)])
                    box["W2E"] = W2E
                return go

            def mk_srow(c, box):
                def go():
                    SrowPS = pp.tile([1, CH], F32, name=f"Srow{c}", tag="EPS")
                    nc.tensor.matmul(
                        SrowPS, lhsT=s_ones, rhs=box["W2E"], start=True, stop=True
                    )
                    box["Srow"] = SrowPS
                return go

            def mk_ln(c, box):
                def go():
                    nc.scalar.activation(
                        out=outsb[:, CH * c : CH * (c + 1)],
                        in_=box["Srow"],
                        func=AF.Ln,
                    )
                return go

            def chunk_body(c, extras):
                EPS = pp.tile([CE, CH], F32, name=f"EPS{c}", tag="EPS")
                dgs = (2 * c, 2 * c + 1)  # groups holding this chunk's diag tiles
                nd = [g for g in range(NG) if g not in dgs]
                # swap the last two so the final four alternate ACT/DVE
                order = nd[:-2] + [nd[-1], nd[-2]] + list(dgs)
                # The E-matmul for group g is emitted one group late so the
                # in-order PE queue never stalls on exp(g): raws of g+1 issue
                # while exp(g) is still in flight.
                nE = [0]

                def emit_E(g, exps, stop):
                    if g == NG - 1:
                        # singleton: tile 65 is zero padding; plain matmul on
                        # tile 64 only (same stream cost as the DR pair)
                        nc.tensor.matmul(
                            EPS,
                            lhsT=s_TAg[:, 2 * CEP * g : 2 * CEP * g + CE],
                            rhs=exps[:, 0:CH],
                            start=(nE[0] == 0),
                            stop=stop,
                        )
                    else:
                        nc.tensor.matmul(
                            EPS,
                            lhsT=s_TAg[:, 2 * CEP * g : 2 * CEP * (g + 1)].rearrange(
                                "p (two c) -> p two c", two=2
                            )[:, :, 0:CE],
                            rhs=exps[:].rearrange("p (two f) -> p two f", two=2),
                            start=(nE[0] == 0),
                            stop=stop,
                            perf_mode=PM.DoubleRow,
                        )
                    nE[0] += 1

                pend = []
                for gi, g in enumerate(order):
                    # group NG-1's second tile (65) is all zero-padding with
                    # all-zero TAg weights: skip its raw+exp; the E-matmul's
                    # zero weights make the stale rhs half harmless.
                    nq = 1 if g == NG - 1 else 2
                    rawPS = rp.tile([128, 2 * CH], F32, name="rawPS", tag="raw")
                    for q in range(nq):
                        t = 2 * g + q
                        dq = t - 4 * c  # 0..3 when t is this chunk's diag tile
                        slot = (2 + 4 * c + dq) if 0 <= dq <= 3 else c
                        nc.tensor.matmul(
                            rawPS[:, CH * q : CH * (q + 1)],
                            lhsT=s_fTg[:, fg_off(t) : fg_off(t) + 256].rearrange(
                                "p (two f) -> p two f", two=2
                            ),
                            rhs=s_fTcz[
                                :, 1024 * slot : 1024 * (slot + 1)
                            ].rearrange("p (two f) -> p two f", two=2),
                            start=True,
                            stop=True,
                            perf_mode=PM.DoubleRow,
                        )
                    exps = ep.tile([128, 2 * CH], E5, name="exps", tag="exps")
                    if g in DVE_G:
                        nc.vector.tensor_scalar(
                            out=exps[:, 0 : nq * CH].bitcast(I8),
                            in0=rawPS[:, 0 : nq * CH],
                            scalar1=A_SLOPE,
                            scalar2=B_CAL,
                            op0=ALU.mult,
                            op1=ALU.add,
                        )
                    else:
                        nc.scalar.activation(
                            out=exps[:, 0 : nq * CH],
                            in_=rawPS[:, 0 : nq * CH],
                            func=AF.Exp,
                            scale=1.0 / TEMP,
                        )
                    pend.append((g, exps))
                    if len(pend) > 2:
                        pg, pe = pend.pop(0)
                        emit_E(pg, pe, stop=False)
                    for fn in extras.pop(gi, ()):
                        fn()
                for fns in extras.values():
                    for fn in fns:
                        fn()
                while pend:
                    pg, pe = pend.pop(0)
                    emit_E(pg, pe, stop=(not pend))
                return EPS

            EPS0 = chunk_body(0, extras={})

            box0 = {}
            extras1 = {
                1: [mk_w2e(0, EPS0, box0)],
                3: [mk_srow(0, box0)],
                5: [mk_ln(0, box0)],
            }
            EPS1 = chunk_body(1, extras=extras1)

            box1 = {}
            mk_w2e(1, EPS1, box1)()
            mk_srow(1, box1)()
            mk_ln(1, box1)()

            nc.sync.dma_start(out=outd[:], in_=outsb)

    nc.finalize()
    return nc


def _get_nc():
    if "nc" not in _NC_CACHE:
        _NC_CACHE["nc"] = _build_nc()
    return _NC_CACHE["nc"]


def _prep_inputs(centers1, features, targets, conf_mask):
    f32 = np.float32
    features = np.ascontiguousarray(features, dtype=f32)
    centers1 = np.ascontiguousarray(centers1, dtype=f32).reshape(-1, D)
    targets = np.ascontiguousarray(targets, dtype=f32)
    conf = np.ascontiguousarray(conf_mask, dtype=f32)

    feats_all = np.concatenate([features, centers1], axis=0)  # [N, D]
    fa = np.zeros((NPAD, D), dtype=f32)
    fa[:N] = feats_all
    q8 = fa.astype(E4NP)  # [NPAD, D] e4m3 (device-exact values)

    labels = targets.argmax(axis=1)
    cc = targets.sum(axis=0, dtype=np.float64) + 1.0  # [C] counts incl. center
    safe = cc > 1.5
    invc = 1.0 / cc
    dcls = np.where(safe, 1.0 / np.maximum(cc - 1.0, 1.0) - invc, 0.0)

    TAe = np.zeros((NPAD, CE), dtype=f32)
    TAe[:B2, :C] = targets
    TAe[B2 : B2 + C, :C] = np.eye(C, dtype=f32)
    TAe[B2 + C, C] = 1.0  # conf-class row: raw==0 -> exp==1.0 exactly

    # host-side positive-pair path (original f32 features, f64 accum)
    f64 = features.astype(np.float64)
    gsum = np.zeros((C, D), np.float64)
    np.add.at(gsum, labels, f64)
    gsum += centers1.astype(np.float64)
    m = cc[labels] - 1.0
    Sm = np.einsum("id,id->i", f64, gsum[labels] - f64)
    numB = float(np.sum(conf * (1.0 / TEMP) / m * Sm))
    den = float(conf.sum())

    eyeq = np.eye(128, dtype=f32).astype(E4NP)

    q8_blocks = q8.reshape(TJ, 128, D)
    TA_blocks = TAe.reshape(TJ, 128, CE)

    in_maps = []
    for c in range(CORES):
        own = list(range(8 * c, 8 * c + 8))
        own_set = set(own)
        order = own + [t for t in range(TJ) if t not in own_set]

        fb = q8_blocks[order]  # [TJ, 128, D]
        fTg_np = np.zeros((D, FGW), dtype=E4NP)
        for t in range(TJ):
            fTg_np[:, fg_off(t) : fg_off(t) + 128] = fb[t].T
        for t in range(8):
            fTg_np[:, fg_off(t) + 128 : fg_off(t) + 256] = eyeq

        TAg_np = np.zeros((128, TJ * CEP), dtype=E5NP)
        tb = TA_blocks[order].transpose(1, 0, 2)  # [128, TJ, CE]
        for t in range(TJ):
            TAg_np[:, CEP * t : CEP * t + CE] = tb[:, t].astype(E5NP)

        rows = slice(c * R, (c + 1) * R)
        fT = q8[rows].T  # [D, R] e4m3, same quantized values as fTg
        fTcz_np = np.zeros((128, 10240), dtype=E4NP)
        fTcz_np[:, 0:512] = fT[:, 0:512]
        fTcz_np[:, 1024:1536] = fT[:, 512:1024]
        for cc_ in (0, 1):
            for qv in range(4):
                base = 1024 * (2 + 4 * cc_ + qv)
                fTcz_np[:, base : base + 512] = fT[:, 512 * cc_ : 512 * (cc_ + 1)]
                blk = fTcz_np[:, base + 512 + 128 * qv : base + 512 + 128 * (qv + 1)]
                np.fill_diagonal(blk.view(np.uint8), np.float32(-2.0).astype(E4NP).view(np.uint8))

        t_ci = targets[rows].T  # [C, R]
        W2C_np = np.zeros((CE, R), dtype=f32)
        W2C_np[:C] = (invc[:, None] + dcls[:, None] * t_ci) * conf[rows][None, :]
        W2C_np[C] = 1.0 - conf[rows]
        W2C_np = W2C_np.astype(BF)

        in_maps.append(
            {
                "fTg": np.ascontiguousarray(fTg_np),
                "fTcz": fTcz_np,
                "TAg": TAg_np,
                "W2C": W2C_np,
            }
        )
    return in_maps, numB, den


def _run(centers1, features, targets, conf_mask, trace=False, trace_cores=None):
    in_maps, numB, den = _prep_inputs(centers1, features, targets, conf_mask)
    nc = _get_nc()
    kwargs = {}
    if trace:
        # NTFF profiling under axon: shim the (absent) antenv.axon_hooks
        # module and skip the artifact bucket upload.
        import types
        import concourse.bass_utils as bass_utils

        if "antenv.axon_hooks" not in sys.modules:
            mod = types.ModuleType("antenv.axon_hooks")
            mod._hook = None

            def set_axon_ntff_profile_hook(h):
                mod._hook = h

            def get_axon_ntff_profile_hook():
                return mod._hook

            mod.set_axon_ntff_profile_hook = set_axon_ntff_profile_hook
            mod.get_axon_ntff_profile_hook = get_axon_ntff_profile_hook
            sys.modules["antenv.axon_hooks"] = mod
            from trn_agent_boot.trn_boot import _ntff_profile_via_ctypes

            set_axon_ntff_profile_hook(
                _ntff_profile_via_ctypes("/opt/axon/libaxon_pjrt.so")
            )
        bass_utils.upload_artifacts = lambda tmpdir: "local://" + tmpdir
        kwargs = {"trace": True}
        if trace_cores is not None:
            kwargs["trace_cores"] = trace_cores
    res = run_bass_kernel_spmd(nc, in_maps, core_ids=list(range(CORES)), **kwargs)
    numA = 0.0
    for r in res.results:
        numA += float(np.asarray(r["out"], np.float64).sum())
    loss = np.array((numA - numB) / den, dtype=np.float32)
    return loss, res


def kernel(centers1, features, targets, cls_num_list, conf_mask):
    loss, _ = _run(centers1, features, targets, conf_mask)
    return loss

